# revision 9
# baseline (speedup 1.0000x reference)
"""Trainium2 Bass kernel: 1D grayscale dilation (max-plus conv) with an
11-tap parabolic structuring element.

  out[i] = max_{j=-5..5} ( x[i+j] + h[j] ),   h[j] = -j^2 / (4*scale)

Exact decomposition per core:
  p_d[i] = max(x[i-d], x[i+d])  via chain  p_d = max(p_{d-1}[-1], p_{d-1}[+1])
  out[i] = max(x[i], max_{d=1..5} (p_d[i] + c_d)),   c_d = -d^2/(4*scale)
(The chain's extra interior points are dominated by closer envelope levels.)

Engine split per tile (T columns x 128 partitions):
  - SWDGE cast-DMA load: HBM fp32 -> SBUF fp16 (halo rows of T+10)
  - DVE: 5 chain pair-maxes + 5 envelope maxes, all fp16 2x-mode aligned
  - ACT: 5 bias-adds (activation Identity + bias tile) + center copy (x + 0)
  - SWDGE cast-DMA store: SBUF fp16 -> HBM fp32

Sharding: 8 contiguous chunks with +-5 halo, one per NeuronCore.
Compute dtype fp16: maxes exact, adds round once -> rel err ~3e-4.
"""

import os
import sys

import numpy as np

for _p in ("/opt/trn_rl_repo", "/root/.axon_site/_ro/trn_rl_repo"):
    if _p not in sys.path and os.path.isdir(_p):
        sys.path.append(_p)

os.environ.setdefault("JAX_COMPILATION_CACHE_DIR", "/tmp/jax_cache")
os.environ.setdefault("JAX_PERSISTENT_CACHE_MIN_COMPILE_TIME_SECS", "1")

import concourse.bacc as bacc
import concourse.mybir as mybir
from bass_rust import AP
from concourse import tile
from concourse.bass_utils import run_bass_kernel_spmd

N = 33554432          # total signal length (2**25)
NCORES = 8
S = N // NCORES       # 4194304 elements per core
HALF = 5              # k//2
ROWS = 128            # SBUF partitions
PER_ROW = S // ROWS   # 32768 elements per partition per core
PAD_VAL = -60000.0    # stand-in for -inf, exactly representable in fp16

F32 = mybir.dt.float32
F16 = mybir.dt.float16
MAX = mybir.AluOpType.max
ADD = mybir.AluOpType.add
IDENT = mybir.ActivationFunctionType.Identity

CFG = {
    "T": 4096,
    "bufs": 2,
    "in_bufs": 3,
    "load_cast": True,    # SWDGE fp32->fp16 cast during load (else HWDGE + DVE cast)
    "store_cast": True,   # SWDGE fp16->fp32 cast during store (else DVE fp32 final op)
    "ts_act": True,       # bias-adds on ACT (else DVE tensor_scalar)
    "repeat": 1,          # loop whole kernel body (timing amplification only)
}

_compiled = {}
LAST_RESULTS = None


def _build(scale_f32: np.float32, cfg=None) -> "bacc.Bacc":
    cfg = {**CFG, **(cfg or {})}
    T = cfg["T"]
    ntiles = PER_ROW // T
    assert PER_ROW % T == 0

    four_scale = np.float32(4.0) * np.float32(scale_f32)
    c = {d: float(np.float32(-(np.float32(d * d)) / four_scale))
         for d in range(1, HALF + 1)}

    nc = bacc.Bacc("TRN2", target_bir_lowering=False, debug=False)
    x = nc.dram_tensor("x", [S + 2 * HALF], F32, kind="ExternalInput")
    out = nc.dram_tensor("out", [S], F32, kind="ExternalOutput")
    x_t = x.ap().tensor
    out2d = out.ap().rearrange("(p m) -> p m", p=ROWS)

    with tile.TileContext(nc) as tc:
        with tc.tile_pool(name="consts", bufs=1) as cpool, \
             tc.tile_pool(name="inpool", bufs=cfg["in_bufs"]) as inpool, \
             tc.tile_pool(name="pool", bufs=cfg["bufs"]) as pool:
            bias = {}
            if cfg["ts_act"]:
                for d in range(0, HALF + 1):
                    bt = cpool.tile([ROWS, 1], F32, tag=f"bias{d}")
                    nc.vector.memset(bt[:, :], c.get(d, 0.0))
                    bias[d] = bt

            def ts_add(out_ap, in_ap, d):
                if cfg["ts_act"]:
                    nc.scalar.activation(out_ap, in_ap, IDENT,
                                         bias=bias[d][:, :], scale=1.0)
                else:
                    nc.vector.tensor_scalar(out_ap, in_ap, c[d], None, op0=ADD)

            import contextlib

            rep_ctx = (tc.For_i(0, cfg["repeat"], 1)
                       if cfg["repeat"] > 1 else contextlib.nullcontext())
            with rep_ctx:
                for i in range(ntiles):
                    # ---- load [128, T+10] with halo (overlapping rows) ----
                    src = AP(tensor=x_t, offset=i * T,
                             ap=[[PER_ROW, ROWS], [1, T + 10]])
                    if cfg["load_cast"]:
                        xin = inpool.tile([ROWS, T + 10], F16, tag="xin")
                        nc.gpsimd.dma_start(out=xin[:, :], in_=src)
                    else:
                        xin32 = inpool.tile([ROWS, T + 10], F32, tag="xin32")
                        nc.sync.dma_start(out=xin32[:, :], in_=src)
                        xin = inpool.tile([ROWS, T + 10], F16, tag="xin")
                        nc.vector.tensor_copy(xin[:, :], xin32[:, :])

                    # ---- chain pair-maxes (DVE, all aligned col-0 bases) ----
                    p = {}
                    prev = xin
                    for d in range(1, HALF + 1):
                        w = T + 10 - 2 * d
                        pd = pool.tile([ROWS, w], F16, tag=f"p{d}")
                        nc.vector.tensor_tensor(pd[:, :], prev[:, 0:w],
                                                prev[:, 2:w + 2], op=MAX)
                        p[d] = pd
                        prev = pd

                    # ---- biased envelope levels ----
                    # center slice of p_d is cols [5-d, 5-d+T)
                    ctr = {d: p[d][:, (HALF - d):(HALF - d) + T]
                           for d in range(1, HALF + 1)}
                    P = {}
                    for d in (1, 3, 5):    # aligned center -> in place
                        ts_add(ctr[d], ctr[d], d)
                        P[d] = ctr[d]
                    for d in (2, 4):       # odd center col -> rebase fresh
                        qd = pool.tile([ROWS, T], F16, tag=f"q{d}")
                        ts_add(qd[:, :], ctr[d], d)
                        P[d] = qd[:, :]

                    # ---- x center term: ACT copy (+0) rebased & cast ----
                    acc = pool.tile([ROWS, T], F16, tag="acc")
                    if cfg["ts_act"]:
                        nc.scalar.activation(acc[:, :], xin[:, HALF:HALF + T],
                                             IDENT, bias=bias[0][:, :], scale=1.0)
                    else:
                        nc.vector.tensor_scalar(acc[:, :], xin[:, HALF:HALF + T],
                                                0.0, None, op0=ADD)

                    # ---- envelope tree (DVE, in-place; depth 3) ----
                    # e1 = max(P1,P3) over P1's slice; e2 = max(P2,P4) over q2
                    nc.vector.tensor_tensor(P[1], P[1], P[3], op=MAX)
                    nc.vector.tensor_tensor(P[2], P[2], P[4], op=MAX)
                    nc.vector.tensor_tensor(acc[:, :], acc[:, :], P[5], op=MAX)
                    nc.vector.tensor_tensor(acc[:, :], acc[:, :], P[1], op=MAX)

                    dst = out2d[:, i * T:(i + 1) * T]
                    if cfg["store_cast"]:
                        nc.vector.tensor_tensor(acc[:, :], acc[:, :], P[2], op=MAX)
                        nc.gpsimd.dma_start(out=dst, in_=acc[:, :])
                    else:
                        ot32 = pool.tile([ROWS, T], F32, tag="ot32")
                        nc.vector.tensor_tensor(ot32[:, :], acc[:, :], P[2], op=MAX)
                        nc.sync.dma_start(out=dst, in_=ot32[:, :])

    nc.compile()
    return nc


def kernel(x: np.ndarray, scale: np.ndarray) -> np.ndarray:
    global LAST_RESULTS
    x = np.asarray(x, dtype=np.float32).reshape(-1)
    assert x.shape[0] == N, f"expected {N} elements, got {x.shape}"
    sv = np.float32(np.asarray(scale).reshape(()))

    key = float(sv)
    if key not in _compiled:
        _compiled[key] = _build(sv)
    nc = _compiled[key]

    xp = np.empty(N + 2 * HALF, dtype=np.float32)
    xp[:HALF] = PAD_VAL
    xp[-HALF:] = PAD_VAL
    xp[HALF:-HALF] = x

    in_maps = [
        {"x": np.ascontiguousarray(xp[cc * S: cc * S + S + 2 * HALF])}
        for cc in range(NCORES)
    ]
    res = run_bass_kernel_spmd(nc, in_maps, core_ids=list(range(NCORES)))
    LAST_RESULTS = res
    out = np.concatenate([np.asarray(res.results[cc]["out"]).reshape(-1)
                          for cc in range(NCORES)])
    return out


if __name__ == "__main__":
    rng = np.random.default_rng(0)
    xs = rng.standard_normal(N).astype(np.float32)
    o = kernel(xs, np.float32(1.5))
    print("out", o.shape, o.dtype, o[:8])
